# revision 17
# baseline (speedup 1.0000x reference)
"""GAT-style attention layer kernel for 8 Trainium2 cores.

Problem: B=16,E=512,DIN=1024,H=8,D=128,T=3 (see harness reference).
Sharding: data-parallel over B (2 batches/core). Math restructured so the
E x E score work runs on PE/ACT:
  left_t = x @ (W_t @ a1_t), right_t = x @ (W_t @ a2_t)   (no full h!)
  scoresT[f,e] = L_t[e] + R_t[f] selected by adj type via +BIG*mask - BIG
  exp_masked = sum_t exp(prelu_0.2(score_t))  (mask folds into exp input)
  outT[d,e] = h_last matmul with exp_masked as rhs; denom via ones-vector
  matmul; normalize+relu fused at the end.
"""
import sys, json

sys.path.insert(0, '/opt/trn_rl_repo')
import numpy as np

B, E, DIN, H, D, T = 16, 512, 1024, 8, 128, 3
NCORES = 8
BPC = B // NCORES          # batches per core
NF = E // 128              # 4 f-tiles
NK = DIN // 128            # 8 k-tiles
BIG = 200.0
SLOPE = 0.2
HT = H * T                 # 24

_cache = {}


def _build():
    import concourse.bass as bass
    import concourse.mybir as mybir
    from concourse import tile

    f32, f32r, bf16 = mybir.dt.float32, mybir.dt.float32r, mybir.dt.bfloat16
    AF = mybir.ActivationFunctionType
    ALU = mybir.AluOpType

    nc = bass.Bass()
    xT_d = nc.declare_dram_parameter("xT", [128, BPC * NK * E], f32, isOutput=False)
    wl_d = nc.declare_dram_parameter("wl", [128, NK * H * D], f32, isOutput=False)
    wt_d = nc.declare_dram_parameter("wt", [128, NK * HT * 128], bf16, isOutput=False)
    a12_d = nc.declare_dram_parameter("a12", [128, HT * 2], bf16, isOutput=False)
    adj_d = nc.declare_dram_parameter("adjT", [128, BPC * NF * E], f32, isOutput=False)
    sel_d = nc.declare_dram_parameter("sel", [128, HT * 128], bf16, isOutput=False)
    inp_d = nc.declare_dram_parameter("inp", [128, 128 + BPC * NF], f32, isOutput=False)
    out_d = nc.declare_dram_parameter("out", [128, BPC * H * E], f32, isOutput=True)

    O_ID, O_NM = 0, 128

    with tile.TileContext(nc) as tc:
        with (
            tc.tile_pool(name="cst", bufs=1) as cst,
            tc.tile_pool(name="stg", bufs=2) as stg,      # DMA staging
            tc.tile_pool(name="sbw", bufs=2) as sbw,      # small working tiles
            tc.tile_pool(name="sc", bufs=3) as sc,        # score-phase lk/ex tiles
        ):
            # ---------------- constants ----------------
            inp = cst.tile([128, 128 + BPC * NF], f32, tag="inp")
            nc.sync.dma_start(out=inp[:], in_=inp_d[:])
            aw = cst.tile([1, 1], f32, tag="aw")
            nc.scalar.activation(aw[:], inp[:1, :1], AF.Copy)
            identf = cst.tile([128, 128], f32, tag="idf")
            nc.vector.tensor_copy(identf[:], inp[:, O_ID:O_ID + 128])
            ident = cst.tile([128, 128], f32r, tag="id")
            nc.vector.tensor_copy(ident[:], identf[:])
            identb = cst.tile([128, 128], bf16, tag="idb")
            nc.vector.tensor_copy(identb[:], identf[:])
            onescol_f = cst.tile([128, 1], f32, tag="ocf")
            nc.vector.memset(onescol_f[:], 1.0)
            onescol = cst.tile([128, 1], f32r, tag="oc")
            nc.vector.tensor_copy(onescol[:], onescol_f[:])
            ones_f = cst.tile([1, 128], f32, tag="onf")
            nc.vector.memset(ones_f[:], 1.0)
            onesb = cst.tile([1, 128], bf16, tag="onb")
            nc.vector.tensor_copy(onesb[:], ones_f[:])

            # ---------------- stream + round xT, WL to f32r ----------------
            xTr = cst.tile([128, BPC, NK, E], f32r, tag="xTr")
            for b in range(BPC):
                for k in range(NK):
                    xs = stg.tile([128, E], f32, tag="xstg")
                    nc.sync.dma_start(out=xs[:], in_=xT_d[:, (b * NK + k) * E:(b * NK + k + 1) * E])
                    nc.vector.tensor_copy(xTr[:, b, k], xs[:])

            # ---------------- w12 (W @ a1|a2), PSUM pool scoped ----------------
            w12sb = cst.tile([128, NK, 48], f32r, tag="w12")
            lr_sb = cst.tile([128, BPC, NF, 48], f32, tag="lrsb")
            lrB = cst.tile([128, BPC, NF, 48], f32, tag="lrB")
            lr_bf = cst.tile([128, BPC, NF, 128], bf16, tag="lrbf")
            nc.vector.memset(lr_bf[:], 0.0)
            lrT_sb = cst.tile([128, BPC, E], bf16, tag="lrT")
            selm = cst.tile([128, HT, 128], bf16, tag="selm")
            nc.sync.dma_start(out=selm[:], in_=sel_d.rearrange("p (a m) -> p a m", a=HT))
            hl_sb = cst.tile([128, BPC, NF, H, D], f32r, tag="hl")

            wlp_outer = tc.tile_pool(name="wlp", bufs=1)
            wlp = wlp_outer.__enter__()
            with tc.tile_pool(name="psW", bufs=2, space="PSUM") as psW:
                wlr = wlp.tile([128, NK, H, D], f32r, tag="wlr")
                for k in range(NK):
                    ws = stg.tile([128, H * D], f32, tag="wstg")
                    nc.sync.dma_start(out=ws[:], in_=wl_d[:, k * H * D:(k + 1) * H * D])
                    nc.vector.tensor_copy(wlr[:, k], ws.rearrange("p (h d) -> p h d", h=H))
                a12 = cst.tile([128, HT, 2], bf16, tag="a12")
                nc.sync.dma_start(out=a12[:], in_=a12_d.rearrange("p (h two) -> p h two", h=HT))
                for k in range(NK):
                    wtk = stg.tile([128, HT * 128], bf16, tag="wtk")
                    nc.sync.dma_start(out=wtk[:], in_=wt_d[:, k * HT * 128:(k + 1) * HT * 128])
                    wtk_v = wtk.rearrange("p (h i) -> p h i", h=HT)
                    w12ps = psW.tile([128, 48], f32, tag="w12ps")
                    for ht in range(HT):
                        nc.tensor.matmul(w12ps[:, 2 * ht:2 * ht + 2], lhsT=wtk_v[:, ht],
                                         rhs=a12[:, ht], start=True, stop=True)
                    nc.scalar.activation(w12sb[:, k], w12ps[:], AF.Identity, bias=0.0, scale=1.0)

                # left/right
                for b in range(BPC):
                    for ec in range(NF):
                        lrps = psW.tile([128, 48], f32, tag="lrps")
                        for k in range(NK):
                            nc.tensor.matmul(lrps[:], lhsT=xTr[:, b, k, ec * 128:(ec + 1) * 128],
                                             rhs=w12sb[:, k], start=(k == 0), stop=(k == NK - 1))
                        nc.scalar.activation(lr_sb[:, b, ec], lrps[:], AF.Identity, bias=0.0, scale=1.0)
                nc.vector.tensor_scalar(lrB[:], lr_sb[:], -BIG, None, op0=ALU.add)
                for b in range(BPC):
                    for ec in range(NF):
                        nc.vector.tensor_copy(lr_bf[:, b, ec, :48], lr_sb[:, b, ec])
                # ACT pre-observe DVE tick for bias APs (wait-slot limit workaround)
                asy = cst.tile([128, BPC * NF * 48], f32, tag="asy")
                nc.scalar.activation(asy[:], lrB.rearrange("p a b c -> p (a b c)"), AF.Copy)

                # L rows: [128e, 128pad] -> [128, 128] PE transposes
                for b in range(BPC):
                    for ec in range(NF):
                        trps = psW.tile([128, 128], bf16, tag="trps")
                        nc.tensor.transpose(trps[:], lr_bf[:, b, ec], identb[:])
                        nc.scalar.activation(lrT_sb[:, b, ec * 128:(ec + 1) * 128], trps[:],
                                             AF.Identity, bias=0.0, scale=1.0)

                # h_last (masked, f32r)
                for b in range(BPC):
                    for hg in range(2):
                        for fc in range(NF):
                            hlps = psW.tile([128, 512], f32, tag="hlps")
                            for k in range(NK):
                                nc.tensor.matmul(hlps[:], lhsT=xTr[:, b, k, fc * 128:(fc + 1) * 128],
                                                 rhs=wlr[:, k, hg * 4:(hg + 1) * 4, :],
                                                 start=(k == 0), stop=(k == NK - 1))
                            nc.scalar.activation(hl_sb[:, b, fc, hg * 4:(hg + 1) * 4, :], hlps[:],
                                                 AF.Identity, bias=0.0,
                                                 scale=inp[:, O_NM + b * NF + fc:O_NM + b * NF + fc + 1])

            # ---------------- score + aggregation per (b, h) ----------------
            with (
                tc.tile_pool(name="psA", bufs=2, space="PSUM") as psA,
                tc.tile_pool(name="psB", bufs=2, space="PSUM") as psB,
                tc.tile_pool(name="psO", bufs=1, space="PSUM") as psO,
                tc.tile_pool(name="mbp", bufs=1) as mbp,
            ):
                for b in range(BPC):
                    # masks for this batch (shared across heads)
                    adjb = mbp.tile([128, NF * E], f32, tag="adjstg")
                    nc.sync.dma_start(out=adjb[:], in_=adj_d[:, b * NF * E:(b + 1) * NF * E])
                    mbig = mbp.tile([128, T, NF, E], bf16, tag="mbig")
                    for t in range(T):
                        for c in range(NF):
                            nc.vector.tensor_scalar(mbig[:, t, c], adjb[:, c * E:(c + 1) * E],
                                                    float(t + 1), BIG, op0=ALU.is_equal, op1=ALU.mult)
                    outsb = mbp.tile([128, H, E], f32, tag="outsb")
                    for h in range(H):
                        outps = psO.tile([128, E], f32, tag="outps")
                        denps = psO.tile([1, E], f32, tag="denps")
                        for c in range(NF):
                            em = psB.tile([128, E], f32, tag="em")
                            for t in range(T):
                                r = (h * 3 + t) * 2
                                s = psA.tile([128, E], f32, tag="s")
                                nc.tensor.matmul(s[:], lhsT=selm[:, h * 3 + t],
                                                 rhs=lrT_sb[:, b], start=True, stop=False)
                                nc.tensor.matmul(s[:], lhsT=identb[:], rhs=mbig[:, t, c],
                                                 start=False, stop=True)
                                lk = sc.tile([128, E], f32, tag="lk")
                                nc.scalar.activation(lk[:], s[:], AF.Prelu,
                                                     bias=lrB[:, b, c, r + 1:r + 2],
                                                     scale=1.0, alpha=SLOPE)
                                ex = sc.tile([128, E], f32r, tag="ex")
                                nc.scalar.activation(ex[:], lk[:], AF.Exp)
                                nc.tensor.matmul(em[:], lhsT=ident[:], rhs=ex[:],
                                                 start=(t == 0), stop=(t == T - 1))
                            emsb = sc.tile([128, E], f32r, tag="emsb")
                            nc.scalar.activation(emsb[:], em[:], AF.Identity, bias=0.0, scale=1.0)
                            nc.tensor.matmul(outps[:], lhsT=hl_sb[:, b, c, h, :], rhs=emsb[:],
                                             start=(c == 0), stop=(c == NF - 1))
                            nc.tensor.matmul(denps[:], lhsT=onescol[:], rhs=emsb[:],
                                             start=(c == 0), stop=(c == NF - 1))
                        den_sb = sbw.tile([1, E], f32, tag="densb")
                        nc.vector.tensor_copy(den_sb[:], denps[:])
                        rec = sbw.tile([1, E], f32, tag="rec")
                        nc.vector.reciprocal(rec[:], den_sb[:])
                        recps = psA.tile([128, E], f32, tag="recps")
                        nc.tensor.matmul(recps[:], lhsT=ones_f[:], rhs=rec[:], start=True, stop=True)
                        recb = sbw.tile([128, E], f32, tag="recb")
                        nc.scalar.activation(recb[:], recps[:], AF.Identity, bias=0.0, scale=1.0)
                        nc.vector.scalar_tensor_tensor(outsb[:, h], in0=outps[:], scalar=0.0,
                                                       in1=recb[:], op0=ALU.max, op1=ALU.mult)
                    nc.sync.dma_start(out=out_d[:, b * H * E:(b + 1) * H * E],
                                      in_=outsb.rearrange("p a b -> p (a b)"))
            wlp_outer.__exit__(None, None, None)

    # --- BIR-JSON drain-wait trim (HW wait-slot limit on tail drain) ---
    _orig = nc.to_json_bytes

    def _patched(out_names=("out",)):
        d = json.loads(_orig())
        keep = set()
        for fn in d.get("functions", []):
            for blk in fn.get("blocks", []):
                for inst in blk.get("instructions", []):
                    if "DMA" in inst.get("opcode", "").upper():
                        outs = inst.get("outs") or []
                        if outs and outs[0].get("memref") in out_names:
                            for u in inst.get("sync_info", {}).get("on_update", []):
                                keep.add(u.get("ant_name"))
        ctr = [0]
        for fn in d.get("functions", []):
            for blk in fn.get("blocks", []):
                out_insts = []
                for inst in blk.get("instructions", []):
                    si = inst.get("sync_info")
                    op = inst.get("opcode", "")
                    if si:
                        w = si.get("on_wait", [])
                        if "DMA" in op.upper() and len(w) > 1:
                            # cross-queue WAR waits dominated by reader-engine wait
                            eng = [x for x in w if "DMA" not in (x.get("ant_name") or "")]
                            if eng:
                                si["on_wait"] = eng[-1:]
                                w = si["on_wait"]
                            if len(w) > 1:
                                si["on_wait"] = w[-1:]
                        elif len(w) > 1:
                            # split excess waits onto same-engine 1-wait drains
                            for extra in w[:-1]:
                                ctr[0] += 1
                                out_insts.append({
                                    "name": f"I-wsplit-{ctr[0]}", "opcode": "Drain",
                                    "engine": inst["engine"], "ins": [], "outs": [],
                                    "is_reset_sema": False,
                                    "debug": inst.get("debug", 0),
                                    "sync_info": {"on_wait": [extra], "on_update": []}})
                            si["on_wait"] = [w[-1]]
                    out_insts.append(inst)
                blk["instructions"] = out_insts
        return json.dumps(d).encode()

    nc.to_json_bytes = _patched
    return nc


def _prep_core(x2, adj2, nm2, W, a1, a2):
    """Host-side relayout for one core holding batches x2=[BPC,E,DIN] etc."""
    import ml_dtypes
    f32 = np.float32
    bf16 = ml_dtypes.bfloat16
    # xT: per b, x[b].T [DIN,E] k-tiled p-major -> [128, BPC*NK*E]
    xt = np.stack([x2[b].T.reshape(NK, 128, E).transpose(1, 0, 2) for b in range(BPC)])
    xT = xt.transpose(1, 0, 2, 3).reshape(128, -1).astype(f32)
    # WL: W[:,T-1] [H,DIN,D] -> [128i, NK, H, D]
    wl = W[:, T - 1].reshape(H, NK, 128, D).transpose(2, 1, 0, 3).reshape(128, -1).astype(f32)
    # WT bf16: [128d, NK, HT, 128i]  (k-major for streaming)
    wt = W.reshape(HT, DIN, 128).transpose(0, 2, 1)          # [HT, 128d, DIN]
    wt = wt.reshape(HT, 128, NK, 128).transpose(1, 2, 0, 3).reshape(128, -1).astype(bf16)
    a12 = np.stack([a1.reshape(HT, 128), a2.reshape(HT, 128)], axis=-1)  # [HT,128d,2]
    a12 = a12.transpose(1, 0, 2).reshape(128, -1).astype(bf16)
    # adjT p-major per b
    adjT = np.stack([adj2[b].T.astype(f32).reshape(NF, 128, E).transpose(1, 0, 2)
                     for b in range(BPC)])
    adjT = adjT.reshape(BPC, 128, -1).transpose(1, 0, 2).reshape(128, -1)
    sel = np.zeros((128, HT, 128), dtype=bf16)
    for idx in range(HT):
        sel[idx * 2, idx, :] = 1
    nm = np.stack([nm2[b, :, 0].astype(f32).reshape(NF, 128).T for b in range(BPC)])
    nm = nm.transpose(1, 0, 2).reshape(128, -1)
    inp = np.concatenate([np.eye(128, dtype=f32), nm], axis=1)
    return {
        "xT": np.ascontiguousarray(xT),
        "wl": np.ascontiguousarray(wl),
        "wt": np.ascontiguousarray(wt),
        "a12": np.ascontiguousarray(a12),
        "adjT": np.ascontiguousarray(adjT),
        "sel": np.ascontiguousarray(sel.reshape(128, -1)),
        "inp": np.ascontiguousarray(inp),
    }


def kernel(x, adj, node_mask, W, a1, a2, _return_results=False, _trace=False):
    from concourse.bass_utils import run_bass_kernel_spmd

    if "nc" not in _cache:
        _cache["nc"] = _build()
    nc = _cache["nc"]

    x = np.asarray(x); adj = np.asarray(adj); node_mask = np.asarray(node_mask)
    W = np.asarray(W); a1 = np.asarray(a1); a2 = np.asarray(a2)

    in_maps = []
    for c in range(NCORES):
        b0 = c * BPC
        in_maps.append(_prep_core(x[b0:b0 + BPC], adj[b0:b0 + BPC],
                                  node_mask[b0:b0 + BPC], W, a1, a2))
    res = run_bass_kernel_spmd(nc, in_maps, list(range(NCORES)), trace=_trace)
    outs = []
    for c in range(NCORES):
        o = res.results[c]["out"]                             # [128d, BPC*H*E]
        o = o.reshape(128, BPC, H, E).transpose(1, 3, 2, 0)   # [b, e, h, d]
        outs.append(o.reshape(BPC, E, H * D))
    full = np.concatenate(outs, axis=0).astype(np.float32)
    if _return_results:
        return full, res
    return full
